# revision 25
# baseline (speedup 1.0000x reference)
"""Multi-LoRA routed adapter kernel for Trainium2 (8 NeuronCores).

Problem: out[b] = (x[b] @ B[aid[b]].T) @ A[aid[b]].T * (alpha/rank)
  x: [8, 1024, 2048] f32, A: [8, 2048, 16] f32, B: [8, 16, 2048] f32,
  adapter_ids: [8] i32, alpha/rank = 16/16 = 1.0.

Strategy: data-parallel over batch — sample b runs on core b. The
adapter gather (routing) is resolved host-side: each core receives only
its sample's selected A/B, pre-transposed so all device DMAs are
contiguous and the contraction dims land on SBUF partitions. All three
inputs are pre-rounded to fp32r (RNE to 11 mantissa bits) host-side so
the matmuls run at the full-rate fp32r path with cast-free HWDGE DMAs.

Per-core device kernel (two chained matmuls over n-chunks of 256):
  matmul1: BxT[r, n]  = sum_i  B^T[i, r]^T  @ xT[i, n]   (K=i tiled by 128)
  matmul2: y[n, o]    = sum_r  BxT[r, n]^T  @ AT[r, o]   (K=r=16)

Overlap structure:
  - loads ride the SP HWDGE ring, stores ride the ACT ring (a store
    waiting on compute would otherwise block later loads in the FIFO);
  - matmul1 of chunk ch+1 is emitted before matmul2 of chunk ch so the
    PE stays dense and the BxT psum->sbuf copy is off the critical path;
  - stores are issued per 128-token slab so the out stream starts as
    soon as the first four psum drains of a chunk land.
"""

import os

import numpy as np

import concourse.bass as bass
import concourse.mybir as mybir
import concourse.tile as tile
from concourse import bacc
from concourse.bass_utils import run_bass_kernel_spmd

# Problem constants (hardcoded per spec).
N_CORES = 8
BATCH = 8
N_TOK = 1024
D_IN = 2048
D_OUT = 2048
RANK = 16
SCALING = 16.0 / 16.0  # alpha / rank

P = 128
K_TILES = D_IN // P  # 16
N_CHUNK = 256  # >=256 keeps fp32r matmul1 at full rate (1 cyc/row)
N_CHUNKS = N_TOK // N_CHUNK
N_SUB = N_CHUNK // P  # 128-token slabs per chunk
O_CHUNK = 512

F32 = mybir.dt.float32
F32R = mybir.dt.float32r

_last_results = None  # stashed BassKernelResults for test harness introspection


def _round_fp32r(a: np.ndarray) -> np.ndarray:
    """Round fp32 to fp32r (sign/8-bit exp/11-bit mantissa), RNE.

    Matches walrus's fp32_to_fp32r: values keep only the top 11 mantissa
    bits; carries may propagate into the exponent (correct RNE).
    """
    u = np.ascontiguousarray(a, dtype=np.float32).view(np.uint32)
    lsb = (u >> np.uint32(12)) & np.uint32(1)
    u = u + np.uint32(0x7FF) + lsb
    u &= np.uint32(0xFFFFF000)
    return u.view(np.float32)


def _build_nc() -> bass.Bass:
    # Bacc (not plain Bass): its compile() pipeline legalizes multi-semaphore
    # waits (move_matmul_waits_to_ldweights, replace_nops_with_events), which
    # walrus requires — raw Tile output can exceed per-instruction wait slots.
    nc = bacc.Bacc(None)
    # xpk[ch, p, kt*N_CHUNK + j] = x[b][ch*N_CHUNK + j, kt*128 + p] — each
    # (chunk, partition) row is 16 KB contiguous, so loads run at line rate.
    xpk = nc.dram_tensor(
        "xpk", [N_CHUNKS, P, K_TILES * N_CHUNK], F32R, kind="ExternalInput"
    )
    BTp = nc.dram_tensor("BTp", [P, K_TILES * RANK], F32R, kind="ExternalInput")
    AT = nc.dram_tensor("AT", [RANK, D_OUT], F32R, kind="ExternalInput")
    y = nc.dram_tensor("y", [N_TOK, D_OUT], F32, kind="ExternalOutput")

    with tile.TileContext(nc) as tc:
        with (
            tc.tile_pool(name="const", bufs=1) as cpool,
            tc.tile_pool(name="xin", bufs=N_CHUNKS) as xpool,
            tc.tile_pool(name="bx", bufs=2) as bxpool,
            tc.tile_pool(name="outb", bufs=3) as opool,
            tc.tile_pool(name="psbx", bufs=2, space="PSUM") as psbx,
            tc.tile_pool(name="pso", bufs=3, space="PSUM") as pso,
        ):
            # All loads up front: they drain back-to-back on the SP ring,
            # and since the stores are enqueued behind them on the SAME
            # FIFO ring, loads get strict priority over stores — the last
            # chunk's load is never starved by out-traffic. The tiny const
            # loads ride between chunk 0 and chunk 1 so the critical first
            # chunk starts draining immediately.
            x_sbs = []
            bt_sb = at_sb = None
            for ch in range(N_CHUNKS):
                x_sb = xpool.tile([P, K_TILES, N_CHUNK], F32R)
                nc.sync.dma_start(
                    x_sb[:], xpk[ch].rearrange("p (kt n) -> p kt n", n=N_CHUNK)
                )
                x_sbs.append(x_sb)
                if ch == 0:
                    bt_sb = cpool.tile([P, K_TILES, RANK], F32R)
                    nc.sync.dma_start(
                        bt_sb[:], BTp.rearrange("p (kt r) -> p kt r", r=RANK)
                    )
                    at_sb = cpool.tile([RANK, D_OUT], F32R)
                    nc.sync.dma_start(at_sb[:], AT[:, :])

            def mm1(ch):
                ps_bx = psbx.tile([RANK, N_CHUNK], F32)
                for kt in range(K_TILES):
                    nc.tensor.matmul(
                        ps_bx[:],
                        bt_sb[:, kt, :],
                        x_sbs[ch][:, kt, :],
                        start=(kt == 0),
                        stop=(kt == K_TILES - 1),
                    )
                bx_sb = bxpool.tile([RANK, N_CHUNK], F32R)
                nc.vector.tensor_copy(bx_sb[:], ps_bx[:])
                return bx_sb

            bx_cur = mm1(0)
            for ch in range(N_CHUNKS):
                bx_next = mm1(ch + 1) if ch + 1 < N_CHUNKS else None
                o_sb = opool.tile([P, N_SUB, D_OUT], F32)
                for ns in range(N_SUB):
                    # Two PSUM banks per tile: one DVE drain covers two
                    # matmuls (halves drain instructions + PE->DVE sems).
                    for op in range(D_OUT // (2 * O_CHUNK)):
                        ps_o = pso.tile([P, 2, O_CHUNK], F32)
                        for half in range(2):
                            oc = 2 * op + half
                            nc.tensor.matmul(
                                ps_o[:, half, :],
                                bx_cur[:, ns * P : (ns + 1) * P],
                                at_sb[:, oc * O_CHUNK : (oc + 1) * O_CHUNK],
                                start=True,
                                stop=True,
                            )
                        nc.vector.tensor_copy(
                            o_sb[:, ns, 2 * op * O_CHUNK : 2 * (op + 1) * O_CHUNK],
                            ps_o[:, :, :],
                        )
                    # Store each 128-token slab (behind the loads, same ring).
                    row0 = ch * N_CHUNK + ns * P
                    nc.sync.dma_start(y[row0 : row0 + P, :], o_sb[:, ns, :])
                bx_cur = bx_next
    nc.compile()
    return nc


def kernel(x, A, B, adapter_ids):
    global _last_results
    x = np.asarray(x, dtype=np.float32)
    A = np.asarray(A, dtype=np.float32)
    B = np.asarray(B, dtype=np.float32)
    adapter_ids = np.asarray(adapter_ids)

    assert x.shape == (BATCH, N_TOK, D_IN)

    in_maps = []
    for b in range(BATCH):
        aid = int(adapter_ids[b])
        # Fold the LoRA scaling into A (scaling is 1.0 here, exact).
        At = _round_fp32r(A[aid].T * np.float32(SCALING))  # [16, 2048]
        # Pack B^T to [p, kt*r]: BTp[p, kt*16+r] = B^T[kt*128+p, r]
        BTp = _round_fp32r(
            B[aid].T.reshape(K_TILES, P, RANK)
            .transpose(1, 0, 2)
            .reshape(P, K_TILES * RANK)
        )
        # [ch, j, kt, p] -> [ch, p, kt, j]: 16 KB contiguous per partition row.
        xpk = _round_fp32r(
            x[b]
            .reshape(N_CHUNKS, N_CHUNK, K_TILES, P)
            .transpose(0, 3, 2, 1)
            .reshape(N_CHUNKS, P, K_TILES * N_CHUNK)
        )
        in_maps.append({"xpk": xpk, "BTp": BTp, "AT": At})

    nc = _build_nc()
    trace = bool(int(os.environ.get("KERNEL_BASS_TRACE", "0")))
    res = run_bass_kernel_spmd(
        nc, in_maps, core_ids=list(range(N_CORES)), trace=trace
    )
    _last_results = res

    out = np.empty((BATCH, N_TOK, D_OUT), dtype=np.float32)
    for b in range(BATCH):
        out[b] = res.results[b]["y"]
    return out


# revision 26
# speedup vs baseline: 1.0099x; 1.0099x over previous
"""Multi-LoRA routed adapter kernel for Trainium2 (8 NeuronCores).

Problem: out[b] = (x[b] @ B[aid[b]].T) @ A[aid[b]].T * (alpha/rank)
  x: [8, 1024, 2048] f32, A: [8, 2048, 16] f32, B: [8, 16, 2048] f32,
  adapter_ids: [8] i32, alpha/rank = 16/16 = 1.0.

Strategy: data-parallel over batch — sample b runs on core b. The
adapter gather (routing) is resolved host-side: each core receives only
its sample's selected A/B, pre-transposed so all device DMAs are
contiguous and the contraction dims land on SBUF partitions. All three
inputs are pre-rounded to fp32r (RNE to 11 mantissa bits) host-side so
the matmuls run at the full-rate fp32r path with cast-free HWDGE DMAs.

Per-core device kernel (two chained matmuls over n-chunks of 256):
  matmul1: BxT[r, n]  = sum_i  B^T[i, r]^T  @ xT[i, n]   (K=i tiled by 128)
  matmul2: y[n, o]    = sum_r  BxT[r, n]^T  @ AT[r, o]   (K=r=16)

Overlap structure:
  - loads ride the SP HWDGE ring, stores ride the ACT ring (a store
    waiting on compute would otherwise block later loads in the FIFO);
  - matmul1 of chunk ch+1 is emitted before matmul2 of chunk ch so the
    PE stays dense and the BxT psum->sbuf copy is off the critical path;
  - stores are issued per 128-token slab so the out stream starts as
    soon as the first four psum drains of a chunk land.
"""

import os

import numpy as np

import concourse.bass as bass
import concourse.mybir as mybir
import concourse.tile as tile
from concourse import bacc
from concourse.bass_utils import run_bass_kernel_spmd

# Problem constants (hardcoded per spec).
N_CORES = 8
BATCH = 8
N_TOK = 1024
D_IN = 2048
D_OUT = 2048
RANK = 16
SCALING = 16.0 / 16.0  # alpha / rank

P = 128
K_TILES = D_IN // P  # 16
N_CHUNK = 256  # >=256 keeps fp32r matmul1 at full rate (1 cyc/row)
N_CHUNKS = N_TOK // N_CHUNK
N_SUB = N_CHUNK // P  # 128-token slabs per chunk
O_CHUNK = 512  # one PSUM bank of fp32 per matmul

F32 = mybir.dt.float32
F32R = mybir.dt.float32r

_last_results = None  # stashed BassKernelResults for test harness introspection


def _round_fp32r(a: np.ndarray) -> np.ndarray:
    """Round fp32 to fp32r (sign/8-bit exp/11-bit mantissa), RNE.

    Matches walrus's fp32_to_fp32r: values keep only the top 11 mantissa
    bits; carries may propagate into the exponent (correct RNE).
    """
    u = np.ascontiguousarray(a, dtype=np.float32).view(np.uint32)
    lsb = (u >> np.uint32(12)) & np.uint32(1)
    u = u + np.uint32(0x7FF) + lsb
    u &= np.uint32(0xFFFFF000)
    return u.view(np.float32)


def _build_nc() -> bass.Bass:
    # Bacc (not plain Bass): its compile() pipeline legalizes multi-semaphore
    # waits (move_matmul_waits_to_ldweights, replace_nops_with_events), which
    # walrus requires — raw Tile output can exceed per-instruction wait slots.
    nc = bacc.Bacc(None)
    # xpk[ch, p, kt*N_CHUNK + j] = x[b][ch*N_CHUNK + j, kt*128 + p] — each
    # (chunk, partition) row is 16 KB contiguous, so loads run at line rate.
    xpk = nc.dram_tensor(
        "xpk", [N_CHUNKS, P, K_TILES * N_CHUNK], F32R, kind="ExternalInput"
    )
    BTp = nc.dram_tensor("BTp", [P, K_TILES * RANK], F32R, kind="ExternalInput")
    AT = nc.dram_tensor("AT", [RANK, D_OUT], F32R, kind="ExternalInput")
    y = nc.dram_tensor("y", [N_TOK, D_OUT], F32, kind="ExternalOutput")

    with tile.TileContext(nc) as tc:
        with (
            tc.tile_pool(name="const", bufs=1) as cpool,
            tc.tile_pool(name="xin", bufs=N_CHUNKS) as xpool,
            tc.tile_pool(name="bx", bufs=2) as bxpool,
            tc.tile_pool(name="outb", bufs=3) as opool,
            tc.tile_pool(name="psbx", bufs=2, space="PSUM") as psbx,
            tc.tile_pool(name="pso", bufs=3, space="PSUM") as pso,
        ):
            # All loads up front: they drain back-to-back on the SP ring,
            # and since the stores are enqueued behind them on the SAME
            # FIFO ring, loads get strict priority over stores — the last
            # chunk's load is never starved by out-traffic. The tiny const
            # loads ride between chunk 0 and chunk 1 so the critical first
            # chunk starts draining immediately.
            x_sbs = []
            bt_sb = at_sb = None
            for ch in range(N_CHUNKS):
                x_sb = xpool.tile([P, K_TILES, N_CHUNK], F32R)
                nc.sync.dma_start(
                    x_sb[:], xpk[ch].rearrange("p (kt n) -> p kt n", n=N_CHUNK)
                )
                x_sbs.append(x_sb)
                if ch == 0:
                    bt_sb = cpool.tile([P, K_TILES, RANK], F32R)
                    nc.sync.dma_start(
                        bt_sb[:], BTp.rearrange("p (kt r) -> p kt r", r=RANK)
                    )
                    at_sb = cpool.tile([RANK, D_OUT], F32R)
                    nc.sync.dma_start(at_sb[:], AT[:, :])

            def mm1(ch):
                ps_bx = psbx.tile([RANK, N_CHUNK], F32)
                for kt in range(K_TILES):
                    nc.tensor.matmul(
                        ps_bx[:],
                        bt_sb[:, kt, :],
                        x_sbs[ch][:, kt, :],
                        start=(kt == 0),
                        stop=(kt == K_TILES - 1),
                    )
                bx_sb = bxpool.tile([RANK, N_CHUNK], F32R)
                nc.vector.tensor_copy(bx_sb[:], ps_bx[:])
                return bx_sb

            bx_cur = mm1(0)
            for ch in range(N_CHUNKS):
                bx_next = mm1(ch + 1) if ch + 1 < N_CHUNKS else None
                o_sb = opool.tile([P, N_SUB, D_OUT], F32)
                for ns in range(N_SUB):
                    # Two PSUM banks per tile: one DVE drain covers two
                    # matmuls (halves drain instructions + PE->DVE sems).
                    for op in range(D_OUT // (2 * O_CHUNK)):
                        ps_o = pso.tile([P, 2, O_CHUNK], F32)
                        for half in range(2):
                            oc = 2 * op + half
                            nc.tensor.matmul(
                                ps_o[:, half, :],
                                bx_cur[:, ns * P : (ns + 1) * P],
                                at_sb[:, oc * O_CHUNK : (oc + 1) * O_CHUNK],
                                start=True,
                                stop=True,
                            )
                        nc.vector.tensor_copy(
                            o_sb[:, ns, 2 * op * O_CHUNK : 2 * (op + 1) * O_CHUNK],
                            ps_o[:, :, :],
                        )
                    # Store each 128-token slab (behind the loads, same ring).
                    row0 = ch * N_CHUNK + ns * P
                    nc.sync.dma_start(y[row0 : row0 + P, :], o_sb[:, ns, :])
                bx_cur = bx_next
    nc.compile()
    return nc


def kernel(x, A, B, adapter_ids):
    global _last_results
    x = np.asarray(x, dtype=np.float32)
    A = np.asarray(A, dtype=np.float32)
    B = np.asarray(B, dtype=np.float32)
    adapter_ids = np.asarray(adapter_ids)

    assert x.shape == (BATCH, N_TOK, D_IN)

    in_maps = []
    for b in range(BATCH):
        aid = int(adapter_ids[b])
        # Fold the LoRA scaling into A (scaling is 1.0 here, exact).
        At = _round_fp32r(A[aid].T * np.float32(SCALING))  # [16, 2048]
        # Pack B^T to [p, kt*r]: BTp[p, kt*16+r] = B^T[kt*128+p, r]
        BTp = _round_fp32r(
            B[aid].T.reshape(K_TILES, P, RANK)
            .transpose(1, 0, 2)
            .reshape(P, K_TILES * RANK)
        )
        # [ch, j, kt, p] -> [ch, p, kt, j]: 16 KB contiguous per partition row.
        xpk = _round_fp32r(
            x[b]
            .reshape(N_CHUNKS, N_CHUNK, K_TILES, P)
            .transpose(0, 3, 2, 1)
            .reshape(N_CHUNKS, P, K_TILES * N_CHUNK)
        )
        in_maps.append({"xpk": xpk, "BTp": BTp, "AT": At})

    nc = _build_nc()
    trace = bool(int(os.environ.get("KERNEL_BASS_TRACE", "0")))
    res = run_bass_kernel_spmd(
        nc, in_maps, core_ids=list(range(N_CORES)), trace=trace
    )
    _last_results = res

    out = np.empty((BATCH, N_TOK, D_OUT), dtype=np.float32)
    for b in range(BATCH):
        out[b] = res.results[b]["y"]
    return out
